# revision 1
# baseline (speedup 1.0000x reference)
"""ConditionalAttentionLayer (gnn_message_passing) Trainium2 kernel.

Sharding: one attention mechanism (head) per NeuronCore, 8 cores.
Each core computes its head's projection h_m = FiLM(x @ W_m), GAT scores,
masked softmax and out_m = attn_m @ h_m over all N=2708 nodes; the host
concatenates the 8 [N, 64] slices into [N, 512].

Math trick used on-device: with s_ij = es_i + ed_j,
  exp(leakyrelu(s)) = max(exp(s), exp(0.2 s))          (exp is monotone)
and softmax over j is invariant to any per-i scale, so dividing by
exp(es_i) gives
  E_ij = adj_ji * max(G_i * D_j, B_j)
with G = exp(-0.8 es), B = exp(ed), D = exp(0.2 ed).  Each [128, 1536]
attention tile is then just one dual-op tensor_scalar (mult+max) and one
masked multiply on DVE — no per-tile transcendentals.  Row sums for the
softmax come free from a ones-column appended to the value matrix in the
PE matmul (output is computed transposed, [65, i], then PE-transposed
back and normalized).
"""

import numpy as np
import ml_dtypes

N = 2708
INS = 1433
OUTS = 64
M = 8
HID = 64

FCH = 12            # feature chunks of 128 (1433 + ones row -> 1536)
FPAD = FCH * 128    # 1536
NB = 22             # node blocks of 128
NPAD = NB * 128     # 2816
NGRP = 2            # i groups
IG = 1408           # i group width (2 * 1408 = 2816 covers N exactly)
IPAD = NGRP * IG    # 2816
SUBS = [(0, 512), (512, 512), (1024, 384)]  # sub-matmul slices per i group
PGRP = [range(0, 6), range(6, 12), range(12, 18), range(18, 22)]

_CACHE = {}


def _build_nc():
    import concourse.bass as bass
    import concourse.mybir as mybir
    import concourse.tile as tile
    from concourse.masks import make_identity

    f32 = mybir.dt.float32
    bf16 = mybir.dt.bfloat16
    Alu = mybir.AluOpType
    Act = mybir.ActivationFunctionType

    nc = bass.Bass("TRN2", use_seq_codegen=True)

    xT = nc.dram_tensor("xT", [FPAD, NPAD], bf16, kind="ExternalInput")
    WWd = nc.dram_tensor("WWd", [FPAD, 128], bf16, kind="ExternalInput")
    adjT = nc.dram_tensor("adjT", [NPAD, IPAD], bf16, kind="ExternalInput")
    aa = nc.dram_tensor("aa", [128, 2, OUTS], f32, kind="ExternalInput")
    wgb = nc.dram_tensor("wgb", [128, 2, OUTS], f32, kind="ExternalInput")
    cst = nc.dram_tensor("cst", [128, 4], f32, kind="ExternalInput")
    out = nc.dram_tensor("out", [N, OUTS], f32, kind="ExternalOutput")

    with tile.TileContext(nc) as tc:
        with (
            tc.tile_pool(name="cpool", bufs=1) as cp,
            tc.tile_pool(name="big", bufs=1) as bigp,
            tc.tile_pool(name="tmp", bufs=2) as tp,
            tc.tile_pool(name="maskp", bufs=3) as mp,
            tc.tile_pool(name="uep", bufs=3) as uep,
            tc.tile_pool(name="dram", bufs=1, space="DRAM") as dp,
        ):
            # ---- constants ----
            aa_s = cp.tile([128, 2, OUTS], f32)
            nc.sync.dma_start(aa_s, aa[:])
            wgb_s = cp.tile([128, 2, OUTS], f32)
            nc.sync.dma_start(wgb_s, wgb[:])
            cst_s = cp.tile([128, 4], f32)
            nc.sync.dma_start(cst_s, cst[:])
            ident = cp.tile([128, 128], f32)
            make_identity(nc, ident)

            es_d = dp.tile([IPAD], f32)

            # ---- resident data (small WW first so matmuls can start on the
            # first xT chunk) ----
            ww_all = bigp.tile([128, FCH, 128], bf16)
            for k in range(FCH):
                nc.sync.dma_start(ww_all[:, k, :], WWd[128 * k:128 * (k + 1), :])
            xt_all = bigp.tile([128, FCH, NPAD], bf16)
            for k in range(FCH):
                nc.sync.dma_start(xt_all[:, k, :], xT[128 * k:128 * (k + 1), :])

            hid_all = bigp.tile([128, NB, OUTS], f32)
            h0_all = bigp.tile([128, NB, OUTS], f32)
            h_all = bigp.tile([128, NB, OUTS + 1], bf16)
            scr4 = bigp.tile([128, NB, 2, OUTS], f32)
            gb_all = bigp.tile([128, NB, 2], f32)
            pq_all = bigp.tile([128, NB, 2], f32)
            ee_all = bigp.tile([128, NB, 2], f32)
            BD = bigp.tile([128, 2, NB], f32)
            g_all = bigp.tile([128, NGRP, IG], bf16)

            nc.vector.memset(h_all[:, :, OUTS:OUTS + 1], 1.0)

            # ---- projection: [h0 | hidden] = x @ [W_m | Wc1] (bias folded) ----
            # k-inner over groups of 6 blocks: matmuls start on the first xT
            # chunk and PE stays dense; conditioner reduce per group hides
            # under the next group's matmuls.
            with tc.tile_pool(name="ppsum", bufs=1, space="PSUM") as pp:
                for blocks in PGRP:
                    hhs = {}
                    for t, b in enumerate(blocks):
                        hhs[b] = pp.tile([128, 128], f32, name=f"hh{t}",
                                         tag=f"hh{t}")
                    for k in range(FCH):
                        for b in blocks:
                            nc.tensor.matmul(
                                hhs[b],
                                lhsT=xt_all[:, k, 128 * b:128 * (b + 1)],
                                rhs=ww_all[:, k, :],
                                start=(k == 0),
                                stop=(k == FCH - 1),
                            )
                    for b in blocks:
                        nc.scalar.activation(hid_all[:, b, :], hhs[b][:, OUTS:128],
                                             Act.Relu)
                        nc.scalar.copy(h0_all[:, b, :], hhs[b][:, 0:OUTS])
                    b0, b1 = blocks.start, blocks.stop
                    nbk = b1 - b0
                    sh4g = (128, nbk, 2, OUTS)
                    nc.vector.tensor_tensor(
                        scr4[:, b0:b1],
                        hid_all[:, b0:b1].unsqueeze(2).to_broadcast(sh4g),
                        wgb_s.unsqueeze(1).to_broadcast(sh4g), Alu.mult)
                    nc.vector.tensor_reduce(
                        gb_all[:, b0:b1], scr4[:, b0:b1],
                        axis=mybir.AxisListType.X, op=Alu.add)
                    nc.vector.tensor_tensor(
                        scr4[:, b0:b1],
                        h0_all[:, b0:b1].unsqueeze(2).to_broadcast(sh4g),
                        aa_s.unsqueeze(1).to_broadcast(sh4g), Alu.mult)
                    nc.vector.tensor_reduce(
                        pq_all[:, b0:b1], scr4[:, b0:b1],
                        axis=mybir.AxisListType.X, op=Alu.add)

                # gamma/beta += bc2; es/ed = gamma*(p,q) + beta*(sum a)
                sh3 = (128, NB, 2)
                nc.vector.tensor_tensor(
                    gb_all, gb_all, cst_s[:, 0:2].unsqueeze(1).to_broadcast(sh3),
                    Alu.add)
                t1 = tp.tile([128, NB, 2], f32, tag="t1")
                nc.vector.tensor_tensor(
                    t1, pq_all, gb_all[:, :, 0:1].to_broadcast(sh3), Alu.mult)
                t2 = tp.tile([128, NB, 2], f32, tag="t2")
                nc.vector.tensor_tensor(
                    t2, gb_all[:, :, 1:2].to_broadcast(sh3),
                    cst_s[:, 2:4].unsqueeze(1).to_broadcast(sh3), Alu.mult)
                nc.vector.tensor_tensor(ee_all, t1, t2, Alu.add)
                # kick the serial es chain first: transpose -> DRAM -> G bcast
                est = pp.tile([NB, 128], f32, name="est", tag="hh0")
                nc.tensor.transpose(est, ee_all[:, :, 0], ident)
                es_sb = tp.tile([NB, 128], f32, tag="es_sb")
                nc.vector.tensor_copy(es_sb, est)
                nc.gpsimd.dma_start(
                    es_d[0:NPAD].rearrange("(b p) -> b p", p=128), es_sb)
                for g in range(NGRP):
                    esb = tp.tile([128, IG], f32, tag="esb")
                    nc.sync.dma_start(
                        esb, es_d[IG * g:IG * (g + 1)].partition_broadcast(128))
                    nc.scalar.activation(g_all[:, g, :], esb, Act.Exp, scale=-0.8)
                # B = exp(ed), D = exp(0.2 ed)
                nc.scalar.activation(BD[:, 0, :], ee_all[:, :, 1], Act.Exp)
                nc.scalar.activation(BD[:, 1, :], ee_all[:, :, 1], Act.Exp,
                                     scale=0.2)
                # FiLM: h = gamma * h0 + beta (overlaps the es DRAM roundtrip)
                shh = (128, NB, OUTS)
                nc.vector.tensor_tensor(
                    h_all[:, :, 0:OUTS], h0_all,
                    gb_all[:, :, 0:1].to_broadcast(shh), Alu.mult)
                nc.vector.tensor_tensor(
                    h_all[:, :, 0:OUTS], h_all[:, :, 0:OUTS],
                    gb_all[:, :, 1:2].to_broadcast(shh), Alu.add)

            # ---- attention ----
            with tc.tile_pool(name="apsum", bufs=1, space="PSUM") as app:
                po = [app.tile([65, IG], f32, name=f"po{g}", tag=f"po{g}")
                      for g in range(NGRP)]
                for j in range(NB):
                    mt = mp.tile([128, IPAD], bf16, tag="mt")
                    nc.sync.dma_start(mt, adjT[128 * j:128 * (j + 1), :])
                    for g in range(NGRP):
                        U = uep.tile([128, IG], bf16, tag="U")
                        nc.vector.tensor_scalar(
                            U, g_all[:, g, :], BD[:, 1, j:j + 1], BD[:, 0, j:j + 1],
                            Alu.mult, Alu.max)
                        E = uep.tile([128, IG], bf16, tag="E")
                        nc.vector.tensor_tensor(E, U, mt[:, IG * g:IG * (g + 1)], Alu.mult)
                        for s0, sw in SUBS:
                            nc.tensor.matmul(
                                po[g][:, s0:s0 + sw],
                                lhsT=h_all[:, j, :],
                                rhs=E[:, s0:s0 + sw],
                                start=(j == 0),
                                stop=(j == NB - 1),
                            )

                # ---- normalize + transpose + store ----
                with tc.tile_pool(name="tpsum", bufs=2, space="PSUM") as tpp:
                    for g in range(NGRP):
                        oT = tp.tile([128, IG], f32, tag="oT")
                        nc.vector.memset(oT[64:128, :], 0.0)
                        nc.scalar.copy(oT[0:65, :], po[g])
                        for t in range(IG // 128):
                            i0 = IG * g + 128 * t
                            if i0 >= N:
                                break
                            v = min(128, N - i0)
                            pt = tpp.tile([128, 128], f32, tag="pt")
                            nc.tensor.transpose(pt, oT[:, 128 * t:128 * (t + 1)], ident)
                            r = tp.tile([128, 1], f32, tag="r", bufs=4)
                            nc.vector.reciprocal(r[:v], pt[:v, OUTS:OUTS + 1])
                            os_ = tp.tile([128, OUTS], f32, tag="os", bufs=4)
                            nc.vector.tensor_scalar_mul(os_[:v], pt[:v, 0:OUTS], r[:v])
                            nc.sync.dma_start(out[i0:i0 + v, :], os_[:v])

    nc.finalize()
    _split_multi_waits(nc, mybir)
    return nc


def _split_multi_waits(nc, mybir):
    """This toolchain's walrus accepts at most one sync wait per HW-decoded
    instruction; hoist extra waits onto standalone EventSemaphore ops on the
    same engine (engines execute their stream in order, so semantics hold)."""
    uid = [0]
    for f in nc.m.functions:
        for bb in f.blocks:
            insts = list(bb.instructions)
            out = []
            changed = False
            for ins in insts:
                si = ins.sync_info
                waits = list(si.on_wait) if si is not None and si.on_wait else []
                if len(waits) > 1:
                    changed = True
                    for w in waits[:-1]:
                        uid[0] += 1
                        ev = mybir.InstEventSemaphore(
                            name=f"splitw_{uid[0]}", ins=[], outs=[])
                        ev.engine = ins.engine
                        ev.sync_info = mybir.SyncInfo(on_wait=[w], on_update=[])
                        out.append(ev)
                    si.on_wait = [waits[-1]]
                out.append(ins)
            if changed:
                bb.instructions = out


def _prep_in_maps(x, adj, W, a_src, a_dst, Wc1, bc1, Wc2, bc2):
    bf = ml_dtypes.bfloat16
    xT_h = np.zeros((FPAD, NPAD), dtype=bf)
    xT_h[:INS, :N] = x.T.astype(bf)
    xT_h[INS, :N] = 1.0  # ones row folds the conditioner bias into the matmul

    adjT_h = np.zeros((NPAD, IPAD), dtype=bf)
    adjT_h[:N, :N] = adj.T.astype(bf)

    in_maps = []
    for m in range(M):
        WW_h = np.zeros((FPAD, 128), dtype=bf)
        WW_h[:INS, 0:OUTS] = W[m].astype(bf)
        WW_h[:INS, OUTS:128] = Wc1.astype(bf)
        WW_h[INS, OUTS:128] = bc1.astype(bf)

        aa_h = np.empty((128, 2, OUTS), dtype=np.float32)
        aa_h[:, 0, :] = a_src[m][None, :]
        aa_h[:, 1, :] = a_dst[m][None, :]

        wgb_h = np.empty((128, 2, OUTS), dtype=np.float32)
        wgb_h[:, 0, :] = Wc2[:, m][None, :]
        wgb_h[:, 1, :] = Wc2[:, M + m][None, :]

        cst_h = np.empty((128, 4), dtype=np.float32)
        cst_h[:, 0] = bc2[m]
        cst_h[:, 1] = bc2[M + m]
        cst_h[:, 2] = float(np.sum(a_src[m], dtype=np.float64))
        cst_h[:, 3] = float(np.sum(a_dst[m], dtype=np.float64))

        in_maps.append({
            "xT": xT_h, "WWd": WW_h, "adjT": adjT_h,
            "aa": aa_h, "wgb": wgb_h, "cst": cst_h,
        })
    return in_maps


def kernel(x, adj, W, a_src, a_dst, Wc1, bc1, Wc2, bc2, _profile=False):
    x = np.asarray(x, dtype=np.float32)
    adj = np.asarray(adj)
    W = np.asarray(W, dtype=np.float32)
    a_src = np.asarray(a_src, dtype=np.float32)
    a_dst = np.asarray(a_dst, dtype=np.float32)
    Wc1 = np.asarray(Wc1, dtype=np.float32)
    bc1 = np.asarray(bc1, dtype=np.float32)
    Wc2 = np.asarray(Wc2, dtype=np.float32)
    bc2 = np.asarray(bc2, dtype=np.float32)

    if "nc" not in _CACHE:
        _CACHE["nc"] = _build_nc()
    nc = _CACHE["nc"]

    from concourse.bass_utils import run_bass_kernel_spmd

    in_maps = _prep_in_maps(x, adj, W, a_src, a_dst, Wc1, bc1, Wc2, bc2)
    res = run_bass_kernel_spmd(
        nc, in_maps, core_ids=list(range(M)), trace=_profile,
    )
    full = np.empty((N, M * OUTS), dtype=np.float32)
    for m in range(M):
        full[:, OUTS * m:OUTS * (m + 1)] = res.results[m]["out"]
    if _profile:
        return full, res
    return full



# revision 16
# speedup vs baseline: 1.0356x; 1.0356x over previous
"""ConditionalAttentionLayer (gnn_message_passing) Trainium2 kernel, v2.

Sharding: one attention mechanism (head) per NeuronCore, 8 cores; host
concatenates the 8 [N, 64] slices.

Per-core pipeline:
  * Projection with stationary weights: hT = [W_m | Wc1]^T x as 12x6
    F=512 matmuls (weights change 12x instead of 264x), PSUM-resident,
    then 22 PE transposes back to node-block layout.
  * Conditioner (gamma/beta/es/ed) via DVE multiply+reduce in block
    layout; es row broadcast across partitions with tiny PE ones-matmuls
    (no DRAM roundtrip).
  * Attention per j-block with E = adj * max(G_i*D_j, B_j) computed as
      T = relu(D_j*G_i - B_j)        (scalar engine, per-partition
                                      scale/bias activation)
      E = (T + B_j) * adj            (one fused DVE scalar_tensor_tensor)
    so the DVE does one pass per mask element instead of two.
  * The adjacency mask lives in HBM as fp8 (exact 0/1) and is cast to
    bf16 during the SWDGE DMA - halves mask HBM traffic while keeping
    the DVE in its 2x 16-bit mode.
  * Output accumulated transposed [65, N] with a ones-column giving the
    softmax denominators; normalized in row form and stored as one
    contiguous [64, N] f32 block (host transposes back).
"""

import numpy as np
import ml_dtypes

N = 2708
INS = 1433
OUTS = 64
M = 8
HID = 64

FCH = 12             # feature chunks of 128 (1433 + ones row -> 1536)
NB = 22              # node blocks of 128
NPAD = NB * 128      # 2816
# i-column chunks for PSUM-bank-sized matmuls over the true N columns
CHUNKS = [(0, 512), (512, 512), (1024, 512), (1536, 512), (2048, 512),
          (2560, 148)]

MASK_FP8 = True      # adjacency as fp8 in HBM, cast to bf16 during DMA

_CACHE = {}


def _build_nc():
    import concourse.bass as bass
    import concourse.mybir as mybir
    import concourse.tile as tile
    from concourse.masks import make_identity

    f32 = mybir.dt.float32
    bf16 = mybir.dt.bfloat16
    f8 = mybir.dt.float8e4
    Alu = mybir.AluOpType
    Act = mybir.ActivationFunctionType

    nc = bass.Bass("TRN2", use_seq_codegen=True)

    xt2 = nc.dram_tensor("xt2", [128, FCH, N], bf16, kind="ExternalInput")
    ww2 = nc.dram_tensor("ww2", [128, FCH, 128], bf16, kind="ExternalInput")
    adj8 = nc.dram_tensor("adj8", [NPAD, N], f8 if MASK_FP8 else bf16,
                          kind="ExternalInput")
    aa = nc.dram_tensor("aa", [128, 2, OUTS], bf16, kind="ExternalInput")
    wgb = nc.dram_tensor("wgb", [128, 2, OUTS], bf16, kind="ExternalInput")
    cst = nc.dram_tensor("cst", [128, 4], f32, kind="ExternalInput")
    out_d = nc.dram_tensor("out", [128, NB, OUTS], f32, kind="ExternalOutput")

    with tile.TileContext(nc) as tc:
        with (
            tc.tile_pool(name="cpool", bufs=1) as cp,
            tc.tile_pool(name="big", bufs=1) as bigp,
            tc.tile_pool(name="tmp", bufs=2) as tp,
            tc.tile_pool(name="maskp", bufs=4) as mp,
            tc.tile_pool(name="selp", bufs=4) as sp,
            tc.tile_pool(name="tpool", bufs=3) as tpl,
            tc.tile_pool(name="epool", bufs=3) as epl,
        ):
            # ---- constants ----
            aa_s = cp.tile([128, 2, OUTS], bf16)
            nc.sync.dma_start(aa_s, aa[:])
            wgb_s = cp.tile([128, 2, OUTS], bf16)
            nc.sync.dma_start(wgb_s, wgb[:])
            cst_s = cp.tile([128, 4], f32)
            nc.sync.dma_start(cst_s, cst[:])
            ident = cp.tile([128, 128], f32)
            make_identity(nc, ident)
            ident_b = cp.tile([128, 128], bf16)
            make_identity(nc, ident_b)


            # ---- resident data ----
            ww_all = bigp.tile([128, FCH, 128], bf16)
            nc.sync.dma_start(ww_all, ww2[:])
            xt_all = bigp.tile([128, FCH, N], bf16)
            for k in range(FCH):
                nc.sync.dma_start(xt_all[:, k, :], xt2[:, k, :])

            cond_sb = bigp.tile([128, N], bf16)     # rows 0:64 h0T, 64:128 hidT
            blocks = bigp.tile([128, NB, 128], bf16)  # [j, block, h0|hid]
            scr4 = bigp.tile([128, NB, 2, OUTS], bf16)
            gb_all = bigp.tile([128, NB, 2], f32)
            pq_all = bigp.tile([128, NB, 2], f32)
            ee_all = bigp.tile([128, NB, 2], f32)
            BD = bigp.tile([128, 3, NB], f32)       # B, D, -B
            h_all = bigp.tile([128, NB, OUTS + 1], bf16)
            g_all = bigp.tile([128, N], bf16)
            est_sb = bigp.tile([NB, 128], f32)

            nc.vector.memset(h_all[:, :, OUTS:OUTS + 1], 1.0)
            nc.vector.memset(blocks[:, NB - 1, :], 0.0)

            # prefetch first masks early (gpsimd stream, SWDGE cast fp8->bf16)
            mts = []
            for j in range(NB):
                mt = mp.tile([128, N], bf16, tag="mt")
                nc.gpsimd.dma_start(mt, adj8[128 * j:128 * (j + 1), :])
                mts.append(mt)

            # ---- projection: hT = [W_m | Wc1]^T @ x, stationary weights ----
            with tc.tile_pool(name="ppsum", bufs=1, space="PSUM") as pp:
                hh = [pp.tile([128, 512], f32, name=f"hh{c}", tag=f"hh{c}")
                      for c in range(6)]
                for k in range(FCH):
                    for c, (c0, cw) in enumerate(CHUNKS):
                        nc.tensor.matmul(
                            hh[c][:, 0:cw],
                            lhsT=ww_all[:, k, :],
                            rhs=xt_all[:, k, c0:c0 + cw],
                            start=(k == 0),
                            stop=(k == FCH - 1),
                        )
                # copy h0T rows + relu(hidden) rows into SBUF (bf16)
                for c, (c0, cw) in enumerate(CHUNKS):
                    nc.scalar.copy(cond_sb[0:OUTS, c0:c0 + cw],
                                   hh[c][0:OUTS, 0:cw])
                    nc.scalar.activation(cond_sb[OUTS:128, c0:c0 + cw],
                                         hh[c][OUTS:128, 0:cw], Act.Relu)

            # ---- transpose back to node-block layout ----
            with tc.tile_pool(name="tps", bufs=4, space="PSUM") as tps:
                for b in range(NB):
                    b0 = 128 * b
                    bw = min(128, N - b0)
                    tb = tps.tile([128, 128], bf16, tag="tb")
                    nc.tensor.transpose(tb[0:bw, :], cond_sb[:, b0:b0 + bw],
                                        ident_b)
                    nc.scalar.copy(blocks[0:bw, b, :], tb[0:bw, :])

                # conditioner reduces: gamma/beta (hid x Wc2), p/q (h0 x a)
                sh4 = (128, NB, 2, OUTS)
                nc.vector.tensor_tensor(
                    scr4, blocks[:, :, OUTS:128].unsqueeze(2).to_broadcast(sh4),
                    wgb_s.unsqueeze(1).to_broadcast(sh4), Alu.mult)
                nc.vector.tensor_reduce(gb_all, scr4, axis=mybir.AxisListType.X,
                                        op=Alu.add)
                nc.vector.tensor_tensor(
                    scr4, blocks[:, :, 0:OUTS].unsqueeze(2).to_broadcast(sh4),
                    aa_s.unsqueeze(1).to_broadcast(sh4), Alu.mult)
                nc.vector.tensor_reduce(pq_all, scr4, axis=mybir.AxisListType.X,
                                        op=Alu.add)

                # gamma/beta += bc2; es/ed = gamma*(p,q) + beta*(sum a)
                sh3 = (128, NB, 2)
                nc.vector.tensor_tensor(
                    gb_all, gb_all,
                    cst_s[:, 0:2].unsqueeze(1).to_broadcast(sh3), Alu.add)
                t1 = tp.tile([128, NB, 2], f32, tag="t1")
                nc.vector.tensor_tensor(
                    t1, pq_all, gb_all[:, :, 0:1].to_broadcast(sh3), Alu.mult)
                t2 = tp.tile([128, NB, 2], f32, tag="t2")
                nc.vector.tensor_tensor(
                    t2, gb_all[:, :, 1:2].to_broadcast(sh3),
                    cst_s[:, 2:4].unsqueeze(1).to_broadcast(sh3), Alu.mult)
                nc.vector.tensor_tensor(ee_all, t1, t2, Alu.add)

                # B = exp(ed), D = exp(0.2 ed), negB = -B
                nc.scalar.activation(BD[:, 0, :], ee_all[:, :, 1], Act.Exp)
                nc.scalar.activation(BD[:, 1, :], ee_all[:, :, 1], Act.Exp,
                                     scale=0.2)
                nc.scalar.activation(BD[:, 2, :], ee_all[:, :, 1], Act.Exp,
                                     scale=1.0)
                nc.vector.tensor_scalar_mul(BD[:, 2, :], BD[:, 2, :], -1.0)

                # FiLM: h = gamma * h0 + beta
                shh = (128, NB, OUTS)
                nc.vector.tensor_tensor(
                    h_all[:, :, 0:OUTS], blocks[:, :, 0:OUTS],
                    gb_all[:, :, 0:1].to_broadcast(shh), Alu.mult)
                nc.vector.tensor_tensor(
                    h_all[:, :, 0:OUTS], h_all[:, :, 0:OUTS],
                    gb_all[:, :, 1:2].to_broadcast(shh), Alu.add)

                # es -> row layout
                est = tps.tile([NB, 128], f32, name="est", tag="est")
                nc.tensor.transpose(est, ee_all[:, :, 0], ident)
                nc.vector.tensor_copy(est_sb, est)

            # ---- broadcast es across partitions, G = exp(-0.8 es) ----
            with tc.tile_pool(name="gps", bufs=1, space="PSUM") as gp:
                gps = gp.tile([128, N], f32)
                for b in range(NB):
                    b0 = 128 * b
                    bw = min(128, N - b0)
                    # selector lhsT: row b all-ones broadcasts est row b
                    selb = sp.tile([NB, 128], f32, tag="sel")
                    nc.gpsimd.memset(selb, 0.0)
                    nc.gpsimd.affine_select(
                        out=selb, in_=selb,
                        compare_op=Alu.not_equal, fill=1.0,
                        base=-b, pattern=[[0, 128]], channel_multiplier=1)
                    nc.tensor.matmul(
                        gps[:, b0:b0 + bw],
                        lhsT=selb,
                        rhs=est_sb[:, 0:bw],
                        start=True, stop=True,
                    )
                nc.scalar.activation(g_all, gps, Act.Exp, scale=-0.8)

            # ---- attention ----
            with tc.tile_pool(name="apsum", bufs=1, space="PSUM") as app:
                po = app.tile([OUTS + 1, N], f32)
                for j in range(NB):
                    T = tpl.tile([128, N], bf16, tag="T")
                    nc.scalar.activation(T, g_all, Act.Relu,
                                         bias=BD[:, 2, j:j + 1],
                                         scale=BD[:, 1, j:j + 1])
                    E = epl.tile([128, N], bf16, tag="E")
                    nc.vector.scalar_tensor_tensor(
                        E, T, BD[:, 0, j:j + 1], mts[j], Alu.add, Alu.mult)
                    for c0, cw in CHUNKS:
                        nc.tensor.matmul(
                            po[:, c0:c0 + cw],
                            lhsT=h_all[:, j, :],
                            rhs=E[:, c0:c0 + cw],
                            start=(j == 0),
                            stop=(j == NB - 1),
                        )

                # ---- normalize via per-block transpose + one p-major store ----
                poc = tp.tile([OUTS + 1, N], bf16, tag="poc", bufs=1)
                nc.scalar.copy(poc, po)
                os_all = tp.tile([128, NB, OUTS], f32, tag="os", bufs=1)
                with tc.tile_pool(name="tpsum", bufs=2, space="PSUM") as tpp:
                    for b in range(NB):
                        b0 = 128 * b
                        bw = min(128, N - b0)
                        pt = tpp.tile([128, OUTS + 1], bf16, tag="pt")
                        nc.tensor.transpose(
                            pt[0:bw, :], poc[:, b0:b0 + bw],
                            ident_b[0:OUTS + 1, 0:OUTS + 1])
                        r = tp.tile([128, 1], f32, tag="r", bufs=4)
                        nc.vector.reciprocal(r[0:bw], pt[0:bw, OUTS:OUTS + 1])
                        nc.vector.tensor_scalar_mul(
                            os_all[0:bw, b, :], pt[0:bw, 0:OUTS], r[0:bw])
                nc.sync.dma_start(out_d[:], os_all)

    nc.finalize()
    _split_multi_waits(nc, mybir)
    return nc


def _split_multi_waits(nc, mybir):
    """This toolchain's walrus accepts at most one sync wait per HW-decoded
    instruction; hoist extra waits onto standalone EventSemaphore ops on the
    same engine (engines execute their stream in order, so semantics hold)."""
    uid = [0]
    for f in nc.m.functions:
        for bb in f.blocks:
            insts = list(bb.instructions)
            out = []
            changed = False
            for ins in insts:
                si = ins.sync_info
                waits = list(si.on_wait) if si is not None and si.on_wait else []
                if len(waits) > 1:
                    changed = True
                    for w in waits[:-1]:
                        uid[0] += 1
                        ev = mybir.InstEventSemaphore(
                            name=f"splitw_{uid[0]}", ins=[], outs=[])
                        ev.engine = ins.engine
                        ev.sync_info = mybir.SyncInfo(on_wait=[w], on_update=[])
                        out.append(ev)
                    si.on_wait = [waits[-1]]
                out.append(ins)
            if changed:
                bb.instructions = out


def _prep_in_maps(x, adj, W, a_src, a_dst, Wc1, bc1, Wc2, bc2):
    bf = ml_dtypes.bfloat16
    f8 = ml_dtypes.float8_e4m3fn

    # xt2[p, k, n] = x[n, 128k+p]; ones row at flat feature index INS
    xt_flat = np.zeros((FCH * 128, N), dtype=bf)
    xt_flat[:INS, :] = x.T.astype(bf)
    xt_flat[INS, :] = 1.0
    xt2_h = np.ascontiguousarray(
        xt_flat.reshape(FCH, 128, N).transpose(1, 0, 2))

    adj_h = np.zeros((NPAD, N), dtype=f8 if MASK_FP8 else bf)
    adj_h[:N, :] = adj.T.astype(adj_h.dtype)

    in_maps = []
    for m in range(M):
        ww_flat = np.zeros((FCH * 128, 128), dtype=bf)
        ww_flat[:INS, 0:OUTS] = W[m].astype(bf)
        ww_flat[:INS, OUTS:128] = Wc1.astype(bf)
        ww_flat[INS, OUTS:128] = bc1.astype(bf)
        ww2_h = np.ascontiguousarray(
            ww_flat.reshape(FCH, 128, 128).transpose(1, 0, 2))

        aa_h = np.empty((128, 2, OUTS), dtype=bf)
        aa_h[:, 0, :] = a_src[m][None, :].astype(bf)
        aa_h[:, 1, :] = a_dst[m][None, :].astype(bf)

        wgb_h = np.empty((128, 2, OUTS), dtype=bf)
        wgb_h[:, 0, :] = Wc2[:, m][None, :].astype(bf)
        wgb_h[:, 1, :] = Wc2[:, M + m][None, :].astype(bf)

        cst_h = np.empty((128, 4), dtype=np.float32)
        cst_h[:, 0] = bc2[m]
        cst_h[:, 1] = bc2[M + m]
        cst_h[:, 2] = float(np.sum(a_src[m], dtype=np.float64))
        cst_h[:, 3] = float(np.sum(a_dst[m], dtype=np.float64))

        in_maps.append({
            "xt2": xt2_h, "ww2": ww2_h, "adj8": adj_h,
            "aa": aa_h, "wgb": wgb_h, "cst": cst_h,
        })
    return in_maps


def kernel(x, adj, W, a_src, a_dst, Wc1, bc1, Wc2, bc2, _profile=False):
    x = np.asarray(x, dtype=np.float32)
    adj = np.asarray(adj)
    W = np.asarray(W, dtype=np.float32)
    a_src = np.asarray(a_src, dtype=np.float32)
    a_dst = np.asarray(a_dst, dtype=np.float32)
    Wc1 = np.asarray(Wc1, dtype=np.float32)
    bc1 = np.asarray(bc1, dtype=np.float32)
    Wc2 = np.asarray(Wc2, dtype=np.float32)
    bc2 = np.asarray(bc2, dtype=np.float32)

    if "nc" not in _CACHE:
        _CACHE["nc"] = _build_nc()
    nc = _CACHE["nc"]

    from concourse.bass_utils import run_bass_kernel_spmd

    in_maps = _prep_in_maps(x, adj, W, a_src, a_dst, Wc1, bc1, Wc2, bc2)
    res = run_bass_kernel_spmd(
        nc, in_maps, core_ids=list(range(M)), trace=_profile,
    )
    full = np.empty((N, M * OUTS), dtype=np.float32)
    for m in range(M):
        o = res.results[m]["out"]  # [128, NB, OUTS], node n = 128*b + p
        full[:, OUTS * m:OUTS * (m + 1)] = (
            o.transpose(1, 0, 2).reshape(NB * 128, OUTS)[:N])
    if _profile:
        return full, res
    return full


# revision 17
# speedup vs baseline: 1.0906x; 1.0531x over previous
"""ConditionalAttentionLayer (gnn_message_passing) Trainium2 kernel, v3.

Sharding: one attention mechanism (head) per NeuronCore, 8 cores; host
concatenates the 8 [N, 64] slices.

Per-core pipeline:
  * Projection with stationary weights: hT = [W_m | Wc1]^T x as 12x6
    F=512 matmuls; xt chunk loads alternate between the two physical
    HWDGE rings (sync + scalar engines) to double DMA issue throughput.
  * Conditioner in node-block layout after 22 PE transposes; the
    multiply+reduce pairs are split into groups of 6 blocks so they
    pipeline with the transposes.
  * es row broadcast across partitions with per-block selector matmuls
    (selector bank built once with one affine_select).
  * Attention: per j-pair one fp8->bf16 cast-DMA (SWDGE) loads two mask
    blocks; per block one dual-op tensor_scalar U = max(G*D_j, B_j)
    (4x-eligible) and per pair one tensor_tensor E = U * adj at 2x.
  * Output accumulated transposed [65, N] with a ones-column giving the
    softmax denominators; per-block transpose + reciprocal normalize
    into a staging tile, one partition-major f32 store.
"""

import numpy as np
import ml_dtypes

N = 2708
INS = 1433
OUTS = 64
M = 8
HID = 64

FCH = 12             # feature chunks of 128 (1433 + ones row -> 1536)
NB = 22              # node blocks of 128
NPAD = NB * 128      # 2816
NPR = 11             # j-block pairs
# i-column chunks for PSUM-bank-sized matmuls over the true N columns
CHUNKS = [(0, 512), (512, 512), (1024, 512), (1536, 512), (2048, 512),
          (2560, 148)]
RGRP = [range(0, 6), range(6, 12), range(12, 18), range(18, 22)]

_CACHE = {}


def _build_nc():
    import concourse.bass as bass
    import concourse.mybir as mybir
    import concourse.tile as tile
    from concourse.masks import make_identity

    f32 = mybir.dt.float32
    bf16 = mybir.dt.bfloat16
    f8 = mybir.dt.float8e4
    Alu = mybir.AluOpType
    Act = mybir.ActivationFunctionType

    nc = bass.Bass("TRN2", use_seq_codegen=True)

    xt2 = nc.dram_tensor("xt2", [128, FCH, N], bf16, kind="ExternalInput")
    ww2 = nc.dram_tensor("ww2", [128, FCH, 128], bf16, kind="ExternalInput")
    adj8 = nc.dram_tensor("adj8", [NPAD, N], f8, kind="ExternalInput")
    aa = nc.dram_tensor("aa", [128, 2, OUTS], bf16, kind="ExternalInput")
    wgb = nc.dram_tensor("wgb", [128, 2, OUTS], bf16, kind="ExternalInput")
    cst = nc.dram_tensor("cst", [128, 4], f32, kind="ExternalInput")
    out_d = nc.dram_tensor("out", [128, NB, OUTS], f32, kind="ExternalOutput")

    with tile.TileContext(nc) as tc:
        with (
            tc.tile_pool(name="cpool", bufs=1) as cp,
            tc.tile_pool(name="big", bufs=1) as bigp,
            tc.tile_pool(name="tmp", bufs=2) as tp,
            tc.tile_pool(name="maskp", bufs=3) as mp,
            tc.tile_pool(name="epool", bufs=3) as epl,
        ):
            # ---- constants ----
            aa_s = cp.tile([128, 2, OUTS], bf16)
            nc.sync.dma_start(aa_s, aa[:])
            wgb_s = cp.tile([128, 2, OUTS], bf16)
            nc.sync.dma_start(wgb_s, wgb[:])
            cst_s = cp.tile([128, 4], f32)
            nc.sync.dma_start(cst_s, cst[:])
            ident_b = cp.tile([128, 128], bf16)
            make_identity(nc, ident_b)
            ident = cp.tile([128, 128], f32)
            make_identity(nc, ident)
            # selector bank: sel[c, b, p] = (c == b); sel[:, b, :] as matmul
            # lhsT broadcasts row b of the rhs to all 128 output partitions
            sel = cp.tile([NB, NB, 128], f32)
            nc.gpsimd.memset(sel, 0.0)
            nc.gpsimd.affine_select(
                out=sel, in_=sel, compare_op=Alu.not_equal, fill=1.0,
                base=0, pattern=[[-1, NB], [0, 128]], channel_multiplier=1)

            # ---- resident data ----
            ww_all = bigp.tile([128, FCH, 128], bf16)
            nc.sync.dma_start(ww_all, ww2[:])
            xt_all = bigp.tile([128, FCH, N], bf16)
            for k in range(FCH):
                eng = nc.sync if k % 2 == 0 else nc.scalar
                eng.dma_start(xt_all[:, k, :], xt2[:, k, :])

            cond_sb = bigp.tile([128, N], bf16)     # rows 0:64 h0T, 64:128 hidT
            blocks = bigp.tile([128, NB, 128], bf16)  # [j, block, h0|hid]
            scr4 = bigp.tile([128, NB, 2, OUTS], bf16)
            gb_all = bigp.tile([128, NB, 2], f32)
            gbb = bigp.tile([128, NB, 2], bf16)
            pq_all = bigp.tile([128, NB, 2], f32)
            ee_all = bigp.tile([128, NB, 2], f32)
            BD = bigp.tile([128, 2, NB], f32)       # B, D
            h_all = bigp.tile([128, NB, OUTS + 1], bf16)
            g_all = bigp.tile([128, N], bf16)
            est_sb = bigp.tile([NB, 128], f32)

            nc.vector.memset(h_all[:, :, OUTS:OUTS + 1], 1.0)
            nc.vector.memset(blocks[:, NB - 1, :], 0.0)

            # mask pair loads (SWDGE cast fp8 -> bf16), 2 blocks per DMA
            mts = []
            for p in range(NPR):
                mt = mp.tile([128, 2, N], bf16, tag="mt")
                nc.gpsimd.dma_start(
                    mt, adj8[256 * p:256 * (p + 1), :].rearrange(
                        "(s q) i -> q s i", q=128))
                mts.append(mt)

            # ---- projection: hT = [W_m | Wc1]^T @ x, stationary weights ----
            with tc.tile_pool(name="ppsum", bufs=1, space="PSUM") as pp:
                hh = [pp.tile([128, 512], f32, name=f"hh{c}", tag=f"hh{c}")
                      for c in range(6)]
                for k in range(FCH):
                    for c, (c0, cw) in enumerate(CHUNKS):
                        nc.tensor.matmul(
                            hh[c][:, 0:cw],
                            lhsT=ww_all[:, k, :],
                            rhs=xt_all[:, k, c0:c0 + cw],
                            start=(k == 0),
                            stop=(k == FCH - 1),
                        )
                # copy h0T rows + relu(hidden) rows into SBUF (bf16)
                for c, (c0, cw) in enumerate(CHUNKS):
                    nc.scalar.copy(cond_sb[0:OUTS, c0:c0 + cw],
                                   hh[c][0:OUTS, 0:cw])
                    nc.scalar.activation(cond_sb[OUTS:128, c0:c0 + cw],
                                         hh[c][OUTS:128, 0:cw], Act.Relu)

            # ---- transpose to node-block layout + grouped reduces ----
            sh3 = (128, NB, 2)
            with tc.tile_pool(name="tps", bufs=4, space="PSUM") as tps:
                for g in RGRP:
                    for b in g:
                        b0 = 128 * b
                        bw = min(128, N - b0)
                        tb = tps.tile([128, 128], bf16, tag="tb")
                        nc.tensor.transpose(tb[0:bw, :],
                                            cond_sb[:, b0:b0 + bw], ident_b)
                        nc.vector.tensor_copy(blocks[0:bw, b, :], tb[0:bw, :])
                    b0, b1 = g.start, g.stop
                    nbk = b1 - b0
                    sh4g = (128, nbk, 2, OUTS)
                    nc.vector.tensor_tensor(
                        scr4[:, b0:b1],
                        blocks[:, b0:b1, OUTS:128].unsqueeze(2).to_broadcast(sh4g),
                        wgb_s.unsqueeze(1).to_broadcast(sh4g), Alu.mult)
                    nc.vector.tensor_reduce(
                        gb_all[:, b0:b1], scr4[:, b0:b1],
                        axis=mybir.AxisListType.X, op=Alu.add)
                    nc.vector.tensor_tensor(
                        scr4[:, b0:b1],
                        blocks[:, b0:b1, 0:OUTS].unsqueeze(2).to_broadcast(sh4g),
                        aa_s.unsqueeze(1).to_broadcast(sh4g), Alu.mult)
                    nc.vector.tensor_reduce(
                        pq_all[:, b0:b1], scr4[:, b0:b1],
                        axis=mybir.AxisListType.X, op=Alu.add)

                # gamma/beta += bc2; es/ed = gamma*(p,q) + beta*(sum a)
                nc.vector.tensor_tensor(
                    gb_all, gb_all,
                    cst_s[:, 0:2].unsqueeze(1).to_broadcast(sh3), Alu.add)
                t1 = tp.tile([128, NB, 2], f32, tag="t1")
                nc.vector.tensor_tensor(
                    t1, pq_all, gb_all[:, :, 0:1].to_broadcast(sh3), Alu.mult)
                t2 = tp.tile([128, NB, 2], f32, tag="t2")
                nc.vector.tensor_tensor(
                    t2, gb_all[:, :, 1:2].to_broadcast(sh3),
                    cst_s[:, 2:4].unsqueeze(1).to_broadcast(sh3), Alu.mult)
                nc.vector.tensor_tensor(ee_all, t1, t2, Alu.add)

                # B = exp(ed), D = exp(0.2 ed)
                nc.scalar.activation(BD[:, 0, :], ee_all[:, :, 1], Act.Exp)
                nc.scalar.activation(BD[:, 1, :], ee_all[:, :, 1], Act.Exp,
                                     scale=0.2)

                # FiLM: h = gamma * h0 + beta  (bf16 gamma/beta for 2x DVE)
                nc.vector.tensor_copy(gbb, gb_all)
                shh = (128, NB, OUTS)
                nc.vector.tensor_tensor(
                    h_all[:, :, 0:OUTS], blocks[:, :, 0:OUTS],
                    gbb[:, :, 0:1].to_broadcast(shh), Alu.mult)
                nc.vector.tensor_tensor(
                    h_all[:, :, 0:OUTS], h_all[:, :, 0:OUTS],
                    gbb[:, :, 1:2].to_broadcast(shh), Alu.add)

                # es -> row layout
                est = tps.tile([NB, 128], f32, name="est", tag="est")
                nc.tensor.transpose(est, ee_all[:, :, 0], ident)
                nc.vector.tensor_copy(est_sb, est)

            # ---- broadcast es across partitions, G = exp(-0.8 es) ----
            with tc.tile_pool(name="gps", bufs=1, space="PSUM") as gp:
                gps = gp.tile([128, N], f32)
                for b in range(NB):
                    b0 = 128 * b
                    bw = min(128, N - b0)
                    nc.tensor.matmul(
                        gps[:, b0:b0 + bw],
                        lhsT=sel[:, b, :],
                        rhs=est_sb[:, 0:bw],
                        start=True, stop=True,
                    )
                nc.scalar.activation(g_all, gps, Act.Exp, scale=-0.8)

            # ---- attention ----
            with tc.tile_pool(name="apsum", bufs=1, space="PSUM") as app:
                po = app.tile([OUTS + 1, N], f32)
                for p in range(NPR):
                    E = epl.tile([128, 2, N], bf16, tag="E")
                    for s in range(2):
                        j = 2 * p + s
                        nc.vector.tensor_scalar(
                            E[:, s, :], g_all,
                            BD[:, 1, j:j + 1], BD[:, 0, j:j + 1],
                            Alu.mult, Alu.max)
                    nc.vector.tensor_tensor(E, E, mts[p], Alu.mult)
                    for s in range(2):
                        j = 2 * p + s
                        for c0, cw in CHUNKS:
                            nc.tensor.matmul(
                                po[:, c0:c0 + cw],
                                lhsT=h_all[:, j, :],
                                rhs=E[:, s, c0:c0 + cw],
                                start=(j == 0),
                                stop=(j == NB - 1),
                            )

                # ---- normalize via per-block transpose + one p-major store ----
                poc = tp.tile([OUTS + 1, N], bf16, tag="poc", bufs=1)
                nc.scalar.copy(poc, po)
                os_all = tp.tile([128, NB, OUTS], f32, tag="os", bufs=1)
                with tc.tile_pool(name="tpsum", bufs=2, space="PSUM") as tpp:
                    for b in range(NB):
                        b0 = 128 * b
                        bw = min(128, N - b0)
                        pt = tpp.tile([128, OUTS + 1], bf16, tag="pt")
                        nc.tensor.transpose(
                            pt[0:bw, :], poc[:, b0:b0 + bw],
                            ident_b[0:OUTS + 1, 0:OUTS + 1])
                        r = tp.tile([128, 1], f32, tag="r", bufs=4)
                        nc.vector.reciprocal(r[0:bw], pt[0:bw, OUTS:OUTS + 1])
                        nc.vector.tensor_scalar_mul(
                            os_all[0:bw, b, :], pt[0:bw, 0:OUTS], r[0:bw])
                nc.sync.dma_start(out_d[:], os_all)

    nc.finalize()
    _split_multi_waits(nc, mybir)
    return nc


def _split_multi_waits(nc, mybir):
    """This toolchain's walrus accepts at most one sync wait per HW-decoded
    instruction; hoist extra waits onto standalone EventSemaphore ops on the
    same engine (engines execute their stream in order, so semantics hold)."""
    uid = [0]
    for f in nc.m.functions:
        for bb in f.blocks:
            insts = list(bb.instructions)
            out = []
            changed = False
            for ins in insts:
                si = ins.sync_info
                waits = list(si.on_wait) if si is not None and si.on_wait else []
                if len(waits) > 1:
                    changed = True
                    for w in waits[:-1]:
                        uid[0] += 1
                        ev = mybir.InstEventSemaphore(
                            name=f"splitw_{uid[0]}", ins=[], outs=[])
                        ev.engine = ins.engine
                        ev.sync_info = mybir.SyncInfo(on_wait=[w], on_update=[])
                        out.append(ev)
                    si.on_wait = [waits[-1]]
                out.append(ins)
            if changed:
                bb.instructions = out


def _prep_in_maps(x, adj, W, a_src, a_dst, Wc1, bc1, Wc2, bc2):
    bf = ml_dtypes.bfloat16
    f8 = ml_dtypes.float8_e4m3fn

    # xt2[p, k, n] = x[n, 128k+p]; ones row at flat feature index INS
    xt_flat = np.zeros((FCH * 128, N), dtype=bf)
    xt_flat[:INS, :] = x.T.astype(bf)
    xt_flat[INS, :] = 1.0
    xt2_h = np.ascontiguousarray(
        xt_flat.reshape(FCH, 128, N).transpose(1, 0, 2))

    adj_h = np.zeros((NPAD, N), dtype=f8)
    adj_h[:N, :] = adj.T.astype(f8)

    in_maps = []
    for m in range(M):
        ww_flat = np.zeros((FCH * 128, 128), dtype=bf)
        ww_flat[:INS, 0:OUTS] = W[m].astype(bf)
        ww_flat[:INS, OUTS:128] = Wc1.astype(bf)
        ww_flat[INS, OUTS:128] = bc1.astype(bf)
        ww2_h = np.ascontiguousarray(
            ww_flat.reshape(FCH, 128, 128).transpose(1, 0, 2))

        aa_h = np.empty((128, 2, OUTS), dtype=bf)
        aa_h[:, 0, :] = a_src[m][None, :].astype(bf)
        aa_h[:, 1, :] = a_dst[m][None, :].astype(bf)

        wgb_h = np.empty((128, 2, OUTS), dtype=bf)
        wgb_h[:, 0, :] = Wc2[:, m][None, :].astype(bf)
        wgb_h[:, 1, :] = Wc2[:, M + m][None, :].astype(bf)

        cst_h = np.empty((128, 4), dtype=np.float32)
        cst_h[:, 0] = bc2[m]
        cst_h[:, 1] = bc2[M + m]
        cst_h[:, 2] = float(np.sum(a_src[m], dtype=np.float64))
        cst_h[:, 3] = float(np.sum(a_dst[m], dtype=np.float64))

        in_maps.append({
            "xt2": xt2_h, "ww2": ww2_h, "adj8": adj_h,
            "aa": aa_h, "wgb": wgb_h, "cst": cst_h,
        })
    return in_maps


def kernel(x, adj, W, a_src, a_dst, Wc1, bc1, Wc2, bc2, _profile=False):
    x = np.asarray(x, dtype=np.float32)
    adj = np.asarray(adj)
    W = np.asarray(W, dtype=np.float32)
    a_src = np.asarray(a_src, dtype=np.float32)
    a_dst = np.asarray(a_dst, dtype=np.float32)
    Wc1 = np.asarray(Wc1, dtype=np.float32)
    bc1 = np.asarray(bc1, dtype=np.float32)
    Wc2 = np.asarray(Wc2, dtype=np.float32)
    bc2 = np.asarray(bc2, dtype=np.float32)

    if "nc" not in _CACHE:
        _CACHE["nc"] = _build_nc()
    nc = _CACHE["nc"]

    from concourse.bass_utils import run_bass_kernel_spmd

    in_maps = _prep_in_maps(x, adj, W, a_src, a_dst, Wc1, bc1, Wc2, bc2)
    res = run_bass_kernel_spmd(
        nc, in_maps, core_ids=list(range(M)), trace=_profile,
    )
    full = np.empty((N, M * OUTS), dtype=np.float32)
    for m in range(M):
        o = res.results[m]["out"]  # [128, NB, OUTS], node n = 128*b + p
        full[:, OUTS * m:OUTS * (m + 1)] = (
            o.transpose(1, 0, 2).reshape(NB * 128, OUTS)[:N])
    if _profile:
        return full, res
    return full
